# revision 36
# baseline (speedup 1.0000x reference)
"""Trainium2 Bass kernel for nn_CrossModalGNNLayer (M=8192, D=128, DEG=32).

out = leaky_relu(local + global + z)
  local[i]  = sum_{k=1..32} alpha[i,k] * wg[(i+k)%M]   (banded GAT attention)
  global    = softmax(z Wq^T Wk z^T / sqrt(d)) @ (z Wc^T)

Sharding: 1024 query rows per core; keys replicated; no collectives.

Dense branch, per 512-query block, streamed over 64 key chunks:
  ST  : bf16 matmul   st[k, q] = G * (z_k . u_q),  u = (Wq^T Wk)^T z^T,
        G = 128*log2(e)/sqrt(d) so st is directly in "bf16 bits" scale.
  exp : split ACT (true exp -> bf16, scale=1/G') and DVE (Schraudolph:
        int16 bits = st + 16250.9 reinterpreted as bf16; |rel err| ~2-3%,
        zero-mean, averages out over 8192 keys).
  PV  : bf16 matmul   h^T[f, q] += zcW_chunk^T @ et,  zcW = z @ Wc^T
        (Wc folded host-side so no separate Wc multiply).
  den : bf16 matmul   den[1, q] += ones^T @ et.
All three matmuls stream 512 moving rows/chunk — the PE floor for this
problem shape.  The banded local branch is software-pipelined into the
chunk loop (12 stages per 128-row group).
"""

import math
import os
import numpy as np
from contextlib import ExitStack

M = 8192
D = 128
DEG = 32
NCORES = 8
ROWS = M // NCORES          # 1024 rows (queries) per core
J = 512                     # query-block size
NB = ROWS // J              # 2 blocks
NCH = M // 128              # 64 key chunks per block
BAND = 160                  # 128 + 32 columns per band block
LEAK = 0.01
SCALE = 1.0 / math.sqrt(D)
A16 = 128.0 / math.log(2.0)   # bf16 bits per ln unit (ST output scale)
A8 = 4.0 / math.log(2.0)      # fp8e5m2 bits per ln unit
B8 = 4.0 * 15 - 4 * 0.0434 + 0.5  # e5m2 schraudolph bias + trunc comp
NPAIR = NCH // 2            # 32 key-chunk pairs per block
LAGP = 2                    # PV/den trail ST by this many pairs

# exp-engine pair split (ACT, DVE) out of 64 chunk-pairs
_EC = os.environ.get("KERNEL_EXP_COUNTS", "34,30")
EXP_COUNTS = tuple(int(x) for x in _EC.split(","))
assert sum(EXP_COUNTS) == 64

_CACHE = {}


def _exp_engine_schedule():
    counts = list(EXP_COUNTS)
    n = len(counts)
    used = [0] * n
    out = []
    for i in range(64):
        e = max(range(n), key=lambda k: counts[k] * (i + 1) / 64.0 - used[k])
        used[e] += 1
        out.append(e)
    return out


def _build_nc():
    import concourse.bass as bass  # noqa: F401
    import concourse.tile as tile
    from concourse import bacc, mybir
    from concourse.masks import make_identity

    f32 = mybir.dt.float32
    bf16 = mybir.dt.bfloat16
    i8 = mybir.dt.int8
    f8e4 = mybir.dt.float8e4
    f8e5 = mybir.dt.float8e5
    f32r = mybir.dt.float32r
    DR = mybir.MatmulPerfMode.DoubleRow
    Act = mybir.ActivationFunctionType
    Alu = mybir.AluOpType

    nc = bacc.Bacc("TRN2", target_bir_lowering=False, debug=False)

    zT = nc.dram_tensor("zT", [D, 2, M], f8e4, kind="ExternalInput")
    uT = nc.dram_tensor("uT", [D, 2, ROWS], f8e4, kind="ExternalInput")
    zcW = nc.dram_tensor("zcW", [128, NPAIR, 2, D], f8e5, kind="ExternalInput")
    wgT = nc.dram_tensor("wgT", [D, 1280], f32r, kind="ExternalInput")
    wgN = nc.dram_tensor("wgN", [128, 10, D], bf16, kind="ExternalInput")
    acB = nc.dram_tensor("acB", [D, 2], f32r, kind="ExternalInput")
    m12d = nc.dram_tensor("m12d", [64, 288], f32r, kind="ExternalInput")
    bmaskB = nc.dram_tensor("bmaskB", [128, BAND], f32, kind="ExternalInput")
    zoc = nc.dram_tensor("zoc", [128, 8, D], f32, kind="ExternalInput")
    out = nc.dram_tensor("out", [ROWS, D], f32, kind="ExternalOutput")

    ENGP = _exp_engine_schedule()

    with tile.TileContext(nc) as tc, ExitStack() as ctx:
        const = ctx.enter_context(tc.tile_pool(name="const", bufs=1))
        big = ctx.enter_context(tc.tile_pool(name="big", bufs=1))
        etp = ctx.enter_context(tc.tile_pool(name="etp", bufs=4))
        bbp = ctx.enter_context(tc.tile_pool(name="bbp", bufs=2))
        ebp = ctx.enter_context(tc.tile_pool(name="ebp", bufs=2))
        aap = ctx.enter_context(tc.tile_pool(name="aap", bufs=2))
        loczp = ctx.enter_context(tc.tile_pool(name="loczp", bufs=4))
        rdbp = ctx.enter_context(tc.tile_pool(name="rdbp", bufs=4))
        rdnp = ctx.enter_context(tc.tile_pool(name="rdnp", bufs=2))
        hsbp = ctx.enter_context(tc.tile_pool(name="hsbp", bufs=2))
        finp = ctx.enter_context(tc.tile_pool(name="finp", bufs=4))
        ps_st = ctx.enter_context(tc.tile_pool(name="ps_st", bufs=1, space="PSUM"))
        ps_h = ctx.enter_context(tc.tile_pool(name="ps_h", bufs=1, space="PSUM"))
        ps_dn = ctx.enter_context(tc.tile_pool(name="ps_dn", bufs=1, space="PSUM"))
        ps_ws = ctx.enter_context(tc.tile_pool(name="ps_ws", bufs=2, space="PSUM"))

        # ---- persistent SBUF ----
        zT_sb = big.tile([D, 2, M], f8e4)
        uT_sb = big.tile([D, 2, ROWS], f8e4)
        zcW_sb = big.tile([128, NPAIR, 2, D], f8e5)
        wgT_sb = big.tile([D, 1280], f32r)
        wgN_sb = big.tile([128, 10, D], bf16)
        m12a = big.tile([64, 288], f32r)
        m12b = big.tile([64, 288], f32r)
        m12 = [m12a, m12b]

        acB_sb = const.tile([D, 2], f32r)
        bm_sb = const.tile([128, BAND], f32)
        zoc_sb = const.tile([128, 8, D], f32)
        ones8 = const.tile([128, 2, 128], f8e5)
        ones_1 = const.tile([1, 1], f32)
        id_bf = const.tile([128, 128], bf16)

        # DMA order: first ST needs uT block-0 half + zT first chunks;
        # first PV needs zcW first pairs; band preproc needs wgT early.
        MS = M // 8
        nc.sync.dma_start(uT_sb[:, :, 0:J], uT[:, :, 0:J])
        nc.sync.dma_start(zT_sb[:, :, 0:256], zT[:, :, 0:256])
        nc.sync.dma_start(zcW_sb[:, 0:1, :, :], zcW[:, 0:1, :, :])
        nc.sync.dma_start(zT_sb[:, :, 256:MS], zT[:, :, 256:MS])
        nc.sync.dma_start(zcW_sb[:, 1:4, :, :], zcW[:, 1:4, :, :])
        nc.sync.dma_start(wgT_sb[:, :], wgT[:, :])
        nc.sync.dma_start(zT_sb[:, :, MS:2 * MS], zT[:, :, MS:2 * MS])
        nc.sync.dma_start(zcW_sb[:, 4:8, :, :], zcW[:, 4:8, :, :])
        nc.sync.dma_start(uT_sb[:, :, J:ROWS], uT[:, :, J:ROWS])
        nc.sync.dma_start(wgN_sb[:, :, :], wgN[:, :, :])
        nc.sync.dma_start(acB_sb[:, :], acB[:, :])
        nc.sync.dma_start(bm_sb[:, :], bmaskB[:, :])
        nc.sync.dma_start(zoc_sb[:, 0:4, :], zoc[:, 0:4, :])
        for s in range(2, 8):
            nc.sync.dma_start(zT_sb[:, :, s * MS:(s + 1) * MS],
                              zT[:, :, s * MS:(s + 1) * MS])
            if s == 3:
                nc.sync.dma_start(zcW_sb[:, 8:16, :, :], zcW[:, 8:16, :, :])
            if s == 5:
                nc.sync.dma_start(zcW_sb[:, 16:24, :, :], zcW[:, 16:24, :, :])
        nc.sync.dma_start(zcW_sb[:, 24:32, :, :], zcW[:, 24:32, :, :])
        nc.sync.dma_start(zoc_sb[:, 4:8, :], zoc[:, 4:8, :])

        nc.vector.memset(ones8[:, :, :], 1.0)
        nc.vector.memset(ones_1[:, :], 1.0)
        make_identity(nc, id_bf[:, :])
        for t in m12:
            nc.sync.dma_start(t[:, :], m12d[:, :])

        def emit_exp(eng, et_sl, st_ps):
            # pair-granular: et_sl [128, 2, 512], st_ps [128, 2, 512]
            if eng == 0:
                nc.scalar.activation(et_sl, st_ps, Act.Exp,
                                     bias=0.0, scale=1.0 / A16)
            else:
                nc.vector.tensor_scalar(et_sl.bitcast(i8), st_ps,
                                        A8 / A16, B8, Alu.mult, Alu.add)

        # ---------- banded local branch, software-pipelined ----------
        band_state = [dict() for _ in range(8)]

        def band_stage(bi, s):
            st = band_state[bi]
            c0 = 128 * bi
            if s == 0:
                ws = ps_ws.tile([128, J], f32, tag="ws")
                st["ws"] = ws
                nc.tensor.matmul(ws[0:1, 0:128], acB_sb[:, 0:1],
                                 wgT_sb[:, c0:c0 + 128],
                                 start=True, stop=True)
                nc.tensor.matmul(ws[0:1, 128:288], acB_sb[:, 1:2],
                                 wgT_sb[:, c0 + 1:c0 + 1 + BAND],
                                 start=True, stop=True)
            elif s == 1:
                m = m12[bi % 2]
                st["m"] = m
                nc.scalar.copy(m[0:1, 0:128], st["ws"][0:1, 0:128])
                nc.vector.tensor_copy(m[32:33, 128:288], st["ws"][0:1, 128:288])
            elif s == 2:
                band_ps = st["ws"][:, 288:288 + BAND]
                st["band_ps"] = band_ps
                m = st["m"]
                nc.tensor.matmul(band_ps, m[:, 0:128], m[:, 128:288],
                                 start=True, stop=True)
            elif s == 3:
                # leaky(x) = 0.01*x + Relu(0.99*x): one PSUM operand per op
                rl = bbp.tile([128, BAND], f32, tag="rl")
                st["rl"] = rl
                nc.scalar.activation(rl[:, :], st["band_ps"], Act.Relu,
                                     bias=0.0, scale=1.0 - LEAK)
            elif s == 4:
                bb = bbp.tile([128, BAND], f32, tag="bb")
                st["bb"] = bb
                nc.vector.scalar_tensor_tensor(bb[:, :], st["band_ps"], LEAK,
                                               st["rl"][:, :],
                                               Alu.mult, Alu.add)
            elif s == 5:
                eb = ebp.tile([128, BAND], bf16, tag="eb")
                dn = rdbp.tile([128, 2], f32, tag="dn")
                st["eb"], st["dn"] = eb, dn
                nc.gpsimd.tensor_tensor(st["bb"][:, :], st["bb"][:, :],
                                        bm_sb[:, :], Alu.add)
            elif s == 6:
                nc.scalar.activation(st["eb"][:, :], st["bb"][:, :], Act.Exp,
                                     bias=0.0, scale=1.0,
                                     accum_out=st["dn"][:, 0:1])
            elif s == 7:
                nc.vector.reciprocal(st["dn"][:, 1:2], st["dn"][:, 0:1])
            elif s == 8:
                ws = st["ws"]
                tr1 = ws[:, 0:64].bitcast(bf16)
                tr2 = ws[0:32, 64:128].bitcast(bf16)
                st["tr1"], st["tr2"] = tr1, tr2
                nc.tensor.transpose(tr1, st["eb"][:, 0:128], id_bf[:, :])
                nc.tensor.transpose(tr2, st["eb"][:, 128:BAND], id_bf[:, :])
            elif s == 9:
                aa = aap.tile([128, 2, 128], bf16, tag="aa")
                st["aa"] = aa
                nc.vector.tensor_copy(aa[:, 0, :], st["tr1"])
                nc.scalar.copy(aa[0:32, 1, :], st["tr2"])
            elif s == 10:
                loc = st["ws"][:, 288:416]
                st["loc"] = loc
                nc.tensor.matmul(loc, st["aa"][:, 0, :], wgN_sb[:, bi, :],
                                 start=True, stop=False)
                nc.tensor.matmul(loc, st["aa"][0:32, 1, :],
                                 wgN_sb[0:32, bi + 1, :],
                                 start=False, stop=True)
            elif s == 11:
                locz = loczp.tile([128, D], f32, tag="locz")
                st["locz"] = locz
                # locz = local_unnorm * (1/band_den) + z
                nc.vector.scalar_tensor_tensor(locz[:, :], st["loc"],
                                               st["dn"][:, 1:2],
                                               zoc_sb[:, bi, :],
                                               Alu.mult, Alu.add)

        BAND_T0 = 3
        BAND_SP = 5           # pair slots between successive bi starts

        def band_tick(gp):
            # global pair slot gp in [0, 64); bi starts at BAND_T0 + SP*bi
            for bi in range(8):
                s = gp - (BAND_T0 + BAND_SP * bi)
                if 0 <= s <= 11:
                    band_stage(bi, s)

        # ---------- dense chunk loop ----------
        stq = ps_st.tile([128, 4, J], f32)  # 4 PSUM banks, manual rotation

        def block(j):
            js = j * J
            h_ps = ps_h.tile([128, J], f32, tag="h")
            dbank = ps_dn.tile([128, J], f32, tag="den")
            ets = {}

            def do_st(p):
                et = etp.tile([128, 2, J], f8e5, tag="et")
                ets[p] = et
                b0 = (2 * p) % 4
                for i in (0, 1):
                    c = 2 * p + i
                    nc.tensor.matmul(stq[:, b0 + i, :],
                                     zT_sb[:, :, c * 128:(c + 1) * 128],
                                     uT_sb[:, :, js:js + J],
                                     start=True, stop=True, perf_mode=DR)
                emit_exp(ENGP[j * NPAIR + p], et[:, :, :],
                         stq[:, b0:b0 + 2, :])

            def do_pv(p):
                et = ets.pop(p)
                first, last = p == 0, p == NPAIR - 1
                nc.tensor.matmul(h_ps[:, :], zcW_sb[:, p, :, :], et[:, :, :],
                                 start=first, stop=last, perf_mode=DR)
                nc.tensor.matmul(dbank[:, :], ones8[:, :, :], et[:, :, :],
                                 start=first, stop=last, perf_mode=DR)

            for p in range(NPAIR + LAGP):
                if p < NPAIR:
                    do_st(p)
                    band_tick(j * NPAIR + p)
                if p >= LAGP:
                    do_pv(p - LAGP)
            return h_ps, dbank

        def finish(j, h_ps, dbank):
            # den [*, 512] -> rden [128, 4] (transpose via tiny matmuls);
            # scratch regions live in the h bank, dead after hsb copies.
            denr = rdnp.tile([1, J], f32, tag="denr")
            nc.vector.tensor_copy(denr[:, :], dbank[0:1, :])
            hsb = hsbp.tile([128, J], bf16, tag="hsb")
            for t in range(4):
                if t % 2 == 0:
                    nc.scalar.copy(hsb[:, t * 128:(t + 1) * 128],
                                   h_ps[:, t * 128:(t + 1) * 128])
                else:
                    nc.vector.tensor_copy(hsb[:, t * 128:(t + 1) * 128],
                                          h_ps[:, t * 128:(t + 1) * 128])
            for t in range(4):
                nc.tensor.matmul(h_ps[:, t:t + 1],
                                 denr[0:1, t * 128:(t + 1) * 128],
                                 ones_1[:, :], start=True, stop=True,
                                 skip_group_check=True)
            rden = rdnp.tile([128, 4], f32, tag="rden")
            nc.vector.reciprocal(rden[:, :], h_ps[:, 0:4])

            for t in range(4):
                bi = j * 4 + t
                locz = band_state[bi]["locz"]
                gtt = h_ps[:, 64 + 64 * t:128 + 64 * t].bitcast(bf16)
                nc.tensor.matmul(gtt, hsb[:, t * 128:(t + 1) * 128],
                                 id_bf[:, :], is_transpose=True,
                                 skip_group_check=True)
                fin = finp.tile([128, 3, D], f32, tag="fin")
                nc.vector.scalar_tensor_tensor(fin[:, 0, :], gtt,
                                               rden[:, t:t + 1], locz[:, :],
                                               Alu.mult, Alu.add)
                nc.scalar.activation(fin[:, 1, :], fin[:, 0, :], Act.Relu,
                                     bias=0.0, scale=1.0 - LEAK)
                nc.vector.scalar_tensor_tensor(fin[:, 2, :], fin[:, 0, :],
                                               LEAK, fin[:, 1, :],
                                               Alu.mult, Alu.add)
                r = j * J + t * 128
                nc.sync.dma_start(out[r:r + 128, :], fin[:, 2, :])

        h0, d0 = block(0)
        finish(0, h0, d0)
        h1, d1 = block(1)
        finish(1, h1, d1)

    nc.compile()
    return nc


def _get_nc():
    if "nc" not in _CACHE:
        _CACHE["nc"] = _build_nc()
    return _CACHE["nc"]


def _bf(x):
    import ml_dtypes
    return np.ascontiguousarray(
        np.asarray(x, np.float32).astype(ml_dtypes.bfloat16))


def _m12_skeleton():
    m = np.zeros((64, 288), np.float32)
    m[32, 0:128] = 1.0   # M1 row32 = ones
    m[0, 128:288] = 1.0  # M2 row0 = ones
    return m


def _f32r(x):
    b = np.ascontiguousarray(np.asarray(x, np.float32)).view(np.uint32)
    b = ((b + 0x800) & np.uint32(0xFFFFF000)).astype(np.uint32)
    return b.view(np.float32)


def _make_in_maps(z, Wg, Wc, Wq, Wk, a):
    z = np.ascontiguousarray(np.asarray(z, dtype=np.float32))
    Wg = np.asarray(Wg, dtype=np.float64)
    Wc = np.asarray(Wc, dtype=np.float64)
    Wq = np.asarray(Wq, dtype=np.float64)
    Wk = np.asarray(Wk, dtype=np.float64)
    a = np.asarray(a, dtype=np.float32)
    zf = z.astype(np.float64)

    import ml_dtypes
    f8 = ml_dtypes.float8_e4m3
    G = A16 * SCALE
    beta = math.sqrt(G)
    B = Wq.T @ Wk
    u = (B.T @ zf.T)                       # [D, M]
    z8 = (beta * zf.T).astype(np.float32).astype(f8)       # [D, M]
    u8 = (beta * u).astype(np.float32).astype(f8)          # [D, M]
    ur8 = (beta * u - u8.astype(np.float64)).astype(np.float32).astype(f8)
    zT_full = np.empty((D, 2, M), dtype=f8)
    zT_full[:, 0, :] = z8
    zT_full[:, 1, :] = z8
    uT_full = np.empty((D, 2, M), dtype=f8)
    uT_full[:, 0, :] = u8
    uT_full[:, 1, :] = ur8

    import ml_dtypes
    zcW = np.asarray(zf @ Wc.T, np.float32).astype(ml_dtypes.float8_e5m2)
    zcW = np.ascontiguousarray(
        zcW.reshape(NPAIR, 2, 128, D).transpose(2, 0, 1, 3))

    wg = zf @ Wg.T                         # [M, D]
    wgT_full = _f32r(wg.T)
    wgN_full = _bf(wg)

    bmask = np.where(
        (np.arange(BAND)[None, :] >= np.arange(128)[:, None])
        & (np.arange(BAND)[None, :] <= np.arange(128)[:, None] + DEG - 1),
        0.0, -30000.0)
    shared = dict(zT=zT_full, zcW=zcW,
                  acB=_f32r(np.stack([a[:D], a[D:]], 1)),
                  m12d=_m12_skeleton(),
                  bmaskB=bmask.astype(np.float32))

    in_maps = []
    for core in range(NCORES):
        r0 = core * ROWS
        uT = np.ascontiguousarray(uT_full[:, :, r0:r0 + ROWS])
        idx = (r0 + np.arange(1280)) % M
        wgT_c = np.ascontiguousarray(wgT_full[:, idx])
        nidx = (r0 + 1 + np.arange(1280)) % M
        wgN_c = np.ascontiguousarray(
            wgN_full[nidx].reshape(10, 128, D).transpose(1, 0, 2))
        zoc = np.ascontiguousarray(
            z[r0:r0 + ROWS].reshape(8, 128, D).transpose(1, 0, 2))
        in_maps.append(dict(shared, uT=uT, wgT=wgT_c, wgN=wgN_c, zoc=zoc))
    return in_maps


def _run(z, Wg, Wc, Wq, Wk, a, trace=False, **kwargs):
    from concourse.bass_utils import run_bass_kernel_spmd
    nc = _get_nc()
    in_maps = _make_in_maps(z, Wg, Wc, Wq, Wk, a)
    res = run_bass_kernel_spmd(nc, in_maps, core_ids=list(range(NCORES)),
                               trace=trace, **kwargs)
    outp = np.concatenate([res.results[i]["out"] for i in range(NCORES)], axis=0)
    return outp.astype(np.float32), res


def _expected_edges(edge_index):
    ei = np.asarray(edge_index).astype(np.int64)
    if ei.shape != (2, M * DEG):
        return False
    src = np.repeat(np.arange(M, dtype=np.int64), DEG)
    dst = (src + np.tile(np.arange(1, DEG + 1, dtype=np.int64), M)) % M
    return bool(np.array_equal(ei[0], src) and np.array_equal(ei[1], dst))


def _leaky(x):
    return np.where(x > 0, x, LEAK * x)


def _numpy_fallback(z, edge_index, Wg, Wc, Wq, Wk, a):
    z = np.asarray(z, dtype=np.float32)
    ei = np.asarray(edge_index).astype(np.int64)
    Wg = np.asarray(Wg, np.float32); Wc = np.asarray(Wc, np.float32)
    Wq = np.asarray(Wq, np.float32); Wk = np.asarray(Wk, np.float32)
    a = np.asarray(a, np.float32)
    m, d = z.shape
    wg = z @ Wg.T
    src, dst = ei[0], ei[1]
    scores = _leaky((wg @ a[:d])[src] + (wg @ a[d:])[dst])
    smax = np.full(m, -np.inf, np.float32)
    np.maximum.at(smax, src, scores)
    ex = np.exp(scores - smax[src])
    denom = np.zeros(m, np.float32)
    np.add.at(denom, src, ex)
    alpha = ex / denom[src]
    local = np.zeros((m, d), np.float32)
    np.add.at(local, src, alpha[:, None] * wg[dst])
    q = z @ Wq.T
    k = z @ Wk.T
    s = (q @ k.T) / np.sqrt(np.float32(d))
    s = s - s.max(axis=-1, keepdims=True)
    e = np.exp(s)
    beta = e / e.sum(axis=-1, keepdims=True)
    gmsg = beta @ (z @ Wc.T)
    return _leaky(local + gmsg + z).astype(np.float32)


def kernel(z, edge_index, Wg, Wc, Wq, Wk, a):
    if not _expected_edges(edge_index):
        return _numpy_fallback(z, edge_index, Wg, Wc, Wq, Wk, a)
    outp, _ = _run(z, Wg, Wc, Wq, Wk, a, trace=False)
    return outp
